# revision 11
# baseline (speedup 1.0000x reference)
"""Trainium2 Bass kernel for nn_Contraction (MACE-style CG contraction).

Math (per node b, channel c):
  wn3 = w_max[elem_b]  (23,C) ; wn2 = w2[elem_b] (5,C) ; wn1 = w1[elem_b] (1,C)
  t[(k,i)]   = wn3[k,c] * x[b,c,i]                        (368)
  c2[wxv]    = sum_ik U3[w,x2,v,i,k] t[(k,i)] + sum_k2 U2[w,x2,v,k2] wn2[k2,c]
  out2[wx2]  = sum_v c2[(w,x2,v)] * x[b,c,v]
  out3[w]    = sum_x2 (out2[(w,x2)] + U1[w,x2,0]*wn1[0,c]) * x[b,c,x2]
  out[b, c*3+w] = out3[w]

Device mapping (per core, Bs=128 nodes, groups of G=4 nodes, F=G*C=512):
  - main matmul, "transposed" orientation: lhsT = U3cat chunks (stationary,
    fixed), rhs = t4 [(k,i)-chunks, (b4,c)] streamed; out1T [(wxv) 6 x 128p,
    (b4,c)] accumulated in PSUM (6 banks). K chunks 128/128/117 (U2 folded as
    rows 112:117 of chunk 2). fp32r at N=512 -> 1 cyc/row.
  - t4 built on GPSIMD: xrep (x bcast over k) * wrep (host-replicated wn3).
  - v-contraction: m6 = out1T * xrep (DVE), then 6 accumulating selector
    matmuls (PE) -> out2T [48, F] in PSUM; rows 48:64 of that PSUM get xT
    (ACT copy) for the U1 path.
  - x2-contraction: m9e = c1_ps * m9in (DVE), one selector matmul with
    U1-extended lhsT -> out3T [3, F].
  - end phase: per-node transposes [3,C] -> [C,3], then 3 big [C,B] -> [B,C]
    transposes into (b, c, w) layout, single contiguous DMA out.

Sharding: data-parallel over nodes b across 8 cores (128 nodes/core).
Host prep (numpy): elem = argmax(y), per-element weight gather/replication,
x transpose, U3/U2 repacking. All FLOPs run on device.
"""

import sys

if "/opt/trn_rl_repo" not in sys.path:
    sys.path.insert(0, "/opt/trn_rl_repo")

import numpy as np

import concourse.bass as bass
import concourse.mybir as mybir
import concourse.tile as tile
from concourse.masks import make_identity

dt = mybir.dt

# problem constants (hardcoded per contract)
B, C, ELL, EQ, E = 1024, 128, 16, 3, 10
P3, P2, P1 = 23, 5, 1
N_CORES = 8
BS = B // N_CORES          # nodes per core
G = 4                      # nodes per group
NG = BS // G               # groups per core
F = G * C                  # streamed free dim (b4, c) = 512
WXV = EQ * ELL * ELL       # 768
WX2 = EQ * ELL             # 48
KTOT = ELL * P3 + P2       # 373
KCH = (128, 128, 112 + P2) # K chunks (chunk2: 112 U3-rows + 5 U2-rows)

_f32 = dt.float32
_f32r = dt.float32r


def _build_program():
    """Build the per-core Bass program (identical across cores)."""
    nc = bass.Bass("TRN2", target_bir_lowering=False, debug=False)

    xrep_d = nc.dram_tensor("xrep", [BS, 128, C], _f32, kind="ExternalInput")
    trep_d = nc.dram_tensor("trep", [3, 128, BS, C], _f32r, kind="ExternalInput")
    wn1_d = nc.dram_tensor("wn1", [BS, C], _f32r, kind="ExternalInput")
    u1w_d = nc.dram_tensor("u1w", [1, WX2], _f32r, kind="ExternalInput")
    u3cat_d = nc.dram_tensor("u3cat", [3, 128, WXV], _f32r, kind="ExternalInput")
    sel6_d = nc.dram_tensor("sel6", [128, 6, WX2], _f32r, kind="ExternalInput")
    sel9_d = nc.dram_tensor("sel9", [WX2, EQ], _f32r, kind="ExternalInput")
    out_d = nc.dram_tensor("out", [BS, C * EQ], _f32, kind="ExternalOutput")

    with tile.TileContext(nc) as tc:
        with tc.tile_pool(name="const", bufs=1) as cpool:
            u3sb = cpool.tile([128, 3, WXV], _f32r)
            nc.sync.dma_start(out=u3sb[:], in_=u3cat_d[:].rearrange("j p f -> p j f"))
            sel6sb = cpool.tile([128, 6, WX2], _f32r)
            nc.sync.dma_start(out=sel6sb[:], in_=sel6_d[:])
            sel9sb = cpool.tile([WX2, EQ], _f32r)
            nc.sync.dma_start(out=sel9sb[:], in_=sel9_d[:])
            u1wsb = cpool.tile([1, WX2], _f32r)
            nc.sync.dma_start(out=u1wsb[:], in_=u1w_d[:])
            outsbT = cpool.tile([EQ, BS * C], _f32)   # [w, (b, c)] staging

            # ---------------- main loop over node groups ----------------
            with tc.tile_pool(name="work", bufs=2) as pool, \
                 tc.tile_pool(name="ps_big", bufs=1, space="PSUM") as psb, \
                 tc.tile_pool(name="ps_c1", bufs=1, space="PSUM") as psc, \
                 tc.tile_pool(name="ps_o3", bufs=1, space="PSUM") as pso:
                for g in range(NG):
                    bsl = slice(g * G, (g + 1) * G)

                    xrep = pool.tile([128, F], _f32, tag="xrep")
                    nc.sync.dma_start(
                        out=xrep[:], in_=xrep_d[bsl].rearrange("b p c -> p b c")
                    )

                    # t4[p,(j,b,c)] = x[b,c,i(p)]*wn3[b,k(j,p),c] (host-built;
                    # chunk2 rows 112:117 carry wn2 for the folded U2 term)
                    t4 = pool.tile([128, 3, F], _f32r, tag="t4")
                    nc.sync.dma_start(
                        out=t4[:], in_=trep_d[:, :, bsl].rearrange("j p b c -> p j b c")
                    )

                    # main matmuls: out1T[(wxv), (b,c)] += U3cat.T @ t4
                    out1_ps = psb.tile([128, 6, F], _f32, tag="out1")
                    for m in range(6):
                        for j in range(3):
                            k = KCH[j]
                            nc.tensor.matmul(
                                out1_ps[:, m, :],
                                u3sb[:k, j, 128 * m : 128 * (m + 1)],
                                t4[:k, j, :],
                                start=(j == 0),
                                stop=(j == 2),
                            )

                    c1_ps = psc.tile([WX2, F], _f32, tag="c1")
                    wn1sb = pool.tile([1, F], _f32r, tag="wn1sb")
                    nc.sync.dma_start(
                        out=wn1sb[:], in_=wn1_d[bsl].unsqueeze(0)
                    )

                    # m6 = out1T * x_v   (v = p % 16)
                    m6 = pool.tile([128, 6, F], _f32r, tag="m6")
                    for h in range(2):
                        nc.vector.tensor_mul(
                            m6[:, 3 * h : 3 * (h + 1), :],
                            out1_ps[:, 3 * h : 3 * (h + 1), :],
                            xrep[:, None, :].to_broadcast([128, 3, F]),
                        )

                    # 6 accumulating selector matmuls + U1*wn1 -> c1 [48, F]
                    for m in range(6):
                        nc.tensor.matmul(
                            c1_ps[:],
                            sel6sb[:, m, :],
                            m6[:, m, :],
                            start=(m == 0),
                            stop=False,
                        )
                    nc.tensor.matmul(
                        c1_ps[:],
                        u1wsb[:],
                        wn1sb[:],
                        start=False,
                        stop=True,
                    )

                    m9e = pool.tile([WX2, F], _f32r, tag="m9e")
                    nc.vector.tensor_mul(m9e[:], c1_ps[:], xrep[:WX2, :])

                    # sel9 (U1-extended) -> out3T [3, F]
                    o3_ps = pso.tile([EQ, F], _f32, tag="o3")
                    nc.tensor.matmul(
                        o3_ps[:],
                        sel9sb[:],
                        m9e[:],
                        start=True,
                        stop=True,
                    )
                    nc.scalar.copy(outsbT[:, g * F : (g + 1) * F], o3_ps[:])

            # ---------------- end phase: layout transform ----------------
            with tc.tile_pool(name="fin", bufs=2) as fpool, \
                 tc.tile_pool(name="ps_fin", bufs=2, space="PSUM") as psf:
                ident3 = cpool.tile([EQ, EQ], _f32)
                make_identity(nc, ident3[:])
                ident128 = cpool.tile([128, 128], _f32)
                make_identity(nc, ident128[:])

                # per-node [3, C] -> [C, 3] into outsb [c, (b, w)]
                outsb = cpool.tile([C, BS * EQ], _f32)
                for g in range(NG):
                    otr_ps = psf.tile([C, G, EQ], _f32, tag="otr")
                    for b in range(G):
                        nc.tensor.transpose(
                            otr_ps[:, b, :],
                            outsbT[:, (g * G + b) * C : (g * G + b + 1) * C],
                            ident3[:],
                        )
                    nc.scalar.copy(
                        outsb[:, g * G * EQ : (g + 1) * G * EQ], otr_ps[:]
                    )

                # [c, (b, w)] -> [b, (c, w)] via 3 big transposes
                finsb = fpool.tile([BS, C * EQ], _f32, tag="finsb")
                outsb_r = outsb[:].rearrange("c (b w) -> c b w", w=EQ)
                finsb_r = finsb[:].rearrange("b (c w) -> b c w", w=EQ)
                for w in range(EQ):
                    fin_ps = psf.tile([BS, C], _f32, tag="fin")
                    nc.tensor.transpose(fin_ps[:], outsb_r[:, :, w], ident128[:])
                    nc.scalar.copy(finsb_r[:, :, w], fin_ps[:])

                nc.sync.dma_start(out=out_d[:], in_=finsb[:])

    # Walrus codegen allows at most one sync-wait per instruction; Tile can
    # emit more. Split them exactly as Bacc.compile does.
    import bass_rust
    bass_rust.move_matmul_waits_to_ldweights(nc.m)
    bass_rust.generate_event_semaphores(nc)
    return nc


def _host_prep(x, y, U3, U2, U1, w_max, w2, w1):
    """Numpy-side input prep: gather per-element weights, transpose x,
    repack U3/U2, build selectors. Returns (shared_consts, per_core_fn)."""
    x = np.ascontiguousarray(x, dtype=np.float32)
    elem = np.argmax(y, axis=1)

    wn3 = w_max[elem]                       # [B, 23, C]
    wn1 = np.ascontiguousarray(w1[elem][:, 0, :])   # [B, C]

    # trep[j, p, b, c] = x[b, c, i(p)] * wn3[b, 8j + p//16, c]; chunk2 rows
    # 112:117 = wn2 (folded U2 contraction operand)
    trep = np.zeros((B, 3, 128, C), dtype=np.float32)
    wn3r = np.repeat(wn3, ELL, axis=1)      # [B, 368, C]
    xtile = np.tile(x.transpose(0, 2, 1), (1, P3, 1))  # [B, 368, C]
    trep.reshape(B, 384, C)[:, :368, :] = wn3r * xtile
    trep[:, 2, 112 : 112 + P2, :] = w2[elem]
    trep = np.ascontiguousarray(trep.transpose(1, 2, 0, 3))  # [3, 128, B, C]

    xT = x.transpose(0, 2, 1)                       # [B, 16, C]
    xrep = np.ascontiguousarray(np.tile(xT, (1, 8, 1)))  # [B, 128, C]

    # U3cat: [k, i, (w, x2, v)] chunks of 128; chunk2 rows 112:117 = U2
    u3k = U3.transpose(4, 3, 0, 1, 2).reshape(ELL * P3, WXV)  # [(k,i), wxv]
    u2k = U2.transpose(3, 0, 1, 2).reshape(P2, WXV)
    u3cat = np.zeros((3, 128, WXV), dtype=np.float32)
    u3cat[0] = u3k[0:128]
    u3cat[1] = u3k[128:256]
    u3cat[2, 0:112] = u3k[256:368]
    u3cat[2, 112 : 112 + P2] = u2k

    sel6 = np.zeros((128, 6, WX2), dtype=np.float32)
    for m in range(6):
        for p in range(128):
            sel6[p, m, 8 * m + p // 16] = 1.0

    sel9 = np.zeros((WX2, EQ), dtype=np.float32)
    for p in range(WX2):
        sel9[p, p // 16] = 1.0
    u1w = np.ascontiguousarray(U1[:, :, 0].reshape(1, WX2))

    shared = {"u3cat": u3cat, "sel6": sel6, "sel9": sel9, "u1w": u1w}

    def per_core(ci):
        s = slice(ci * BS, (ci + 1) * BS)
        m = {
            "xrep": np.ascontiguousarray(xrep[s]),
            "trep": np.ascontiguousarray(trep[:, :, s]),
            "wn1": np.ascontiguousarray(wn1[s]),
        }
        m.update(shared)
        return m

    return per_core


_PROGRAM_CACHE = {}


def kernel(**inputs) -> np.ndarray:
    from concourse.bass_utils import run_bass_kernel_spmd

    per_core = _host_prep(
        np.asarray(inputs["x"]), np.asarray(inputs["y"]),
        np.asarray(inputs["U3"]), np.asarray(inputs["U2"]),
        np.asarray(inputs["U1"]), np.asarray(inputs["w_max"]),
        np.asarray(inputs["w2"]), np.asarray(inputs["w1"]),
    )

    if "nc" not in _PROGRAM_CACHE:
        _PROGRAM_CACHE["nc"] = _build_program()
    nc = _PROGRAM_CACHE["nc"]

    in_maps = [per_core(ci) for ci in range(N_CORES)]
    res = run_bass_kernel_spmd(nc, in_maps, core_ids=list(range(N_CORES)))
    out = np.concatenate([r["out"] for r in res.results], axis=0)
    return out.astype(np.float32)


if __name__ == "__main__":
    # smoke test in CoreSim on core 0's shard
    from concourse.bass_interp import CoreSim

    rng = np.random.default_rng(0)
    x = rng.standard_normal((B, C, ELL)).astype(np.float32)
    elem = rng.integers(0, E, size=B)
    y = np.eye(E, dtype=np.float32)[elem]
    U3 = (rng.standard_normal((EQ, ELL, ELL, ELL, P3)) * 0.1).astype(np.float32)
    U2 = (rng.standard_normal((EQ, ELL, ELL, P2)) * 0.1).astype(np.float32)
    U1 = (rng.standard_normal((EQ, ELL, P1)) * 0.1).astype(np.float32)
    w_max = (rng.standard_normal((E, P3, C)) / P3).astype(np.float32)
    w2 = (rng.standard_normal((E, P2, C)) / P2).astype(np.float32)
    w1 = (rng.standard_normal((E, P1, C)) / P1).astype(np.float32)

    per_core = _host_prep(x, y, U3, U2, U1, w_max, w2, w1)
    nc = _build_program()
    sim = CoreSim(nc)
    m = per_core(0)
    for k, v in m.items():
        sim.tensor(k)[:] = v
    sim.simulate(check_with_hw=False, trace_hw=False)
    got = np.array(sim.tensor("out"))

    # numpy reference for core 0 shard
    def ref_np(x, y, U3, U2, U1, w_max, w2, w1):
        wn3 = np.einsum("be,ekc->bkc", y, w_max)
        t = np.einsum("bkc,bci->bcik", wn3, x)
        out = np.einsum("wxvik,bcik->bcwxv", U3, t)
        wn2 = np.einsum("be,ekc->bkc", y, w2)
        c2 = np.einsum("wxvk,bkc->bcwxv", U2, wn2) + out
        out = np.einsum("bcwxi,bci->bcwx", c2, x)
        wn1 = np.einsum("be,ekc->bkc", y, w1)
        c1 = np.einsum("wxk,bkc->bcwx", U1, wn1) + out
        out = np.einsum("bcwi,bci->bcw", c1, x)
        return out.reshape(out.shape[0], -1)

    want = ref_np(x[:BS], y[:BS], U3, U2, U1, w_max, w2, w1)
    err = np.abs(got - want).max() / (np.abs(want).max() + 1e-30)
    print(f"CoreSim vs numpy rel err: {err:.3e}")
    assert err < 2e-2, "FAIL"
    print("SIM PASS")


# revision 13
# speedup vs baseline: 2.6630x; 2.6630x over previous
"""Trainium2 Bass kernel for nn_Contraction (MACE-style CG contraction).

Math (per node b, channel c):
  wn3 = w_max[elem_b]  (23,C) ; wn2 = w2[elem_b] (5,C) ; wn1 = w1[elem_b] (1,C)
  t[(k,i)]   = wn3[k,c] * x[b,c,i]                        (368)
  c2[wxv]    = sum_ik U3[w,x2,v,i,k] t[(k,i)] + sum_k2 U2[w,x2,v,k2] wn2[k2,c]
  out2[wx2]  = sum_v c2[(w,x2,v)] * x[b,c,v]
  out3[w]    = sum_x2 (out2[(w,x2)] + U1[w,x2,0]*wn1[0,c]) * x[b,c,x2]
  out[b, c*3+w] = out3[w]

Device mapping (per core, Bs=128 nodes, groups of G=4 nodes, F=G*C=512):
  - main matmul, "transposed" orientation: lhsT = U3cat chunks (stationary,
    fixed), rhs = t4 [(k,i)-chunks, (b4,c)] streamed; out1T [(wxv) 6 x 128p,
    (b4,c)] accumulated in PSUM (6 banks). K chunks 128/128/117 (U2 folded as
    rows 112:117 of chunk 2). fp32r at N=512 -> 1 cyc/row.
  - t4 built on GPSIMD: xrep (x bcast over k) * wrep (host-replicated wn3).
  - v-contraction: m6 = out1T * xrep (DVE), then 6 accumulating selector
    matmuls (PE) -> out2T [48, F] in PSUM; rows 48:64 of that PSUM get xT
    (ACT copy) for the U1 path.
  - x2-contraction: m9e = c1_ps * m9in (DVE), one selector matmul with
    U1-extended lhsT -> out3T [3, F].
  - end phase: per-node transposes [3,C] -> [C,3], then 3 big [C,B] -> [B,C]
    transposes into (b, c, w) layout, single contiguous DMA out.

Sharding: data-parallel over nodes b across 8 cores (128 nodes/core).
Host prep (numpy): elem = argmax(y), per-element weight gather/replication,
x transpose, U3/U2 repacking. All FLOPs run on device.
"""

import sys

if "/opt/trn_rl_repo" not in sys.path:
    sys.path.insert(0, "/opt/trn_rl_repo")

import numpy as np

import concourse.bass as bass
import concourse.mybir as mybir
import concourse.tile as tile
from concourse.masks import make_identity

dt = mybir.dt

# problem constants (hardcoded per contract)
B, C, ELL, EQ, E = 1024, 128, 16, 3, 10
P3, P2, P1 = 23, 5, 1
N_CORES = 8
BS = B // N_CORES          # nodes per core
G = 4                      # nodes per group
NG = BS // G               # groups per core
F = G * C                  # streamed free dim (b4, c) = 512
WXV = EQ * ELL * ELL       # 768
WX2 = EQ * ELL             # 48
KTOT = ELL * P3 + P2       # 373
KCH = (128, 128, 112 + P2) # K chunks (chunk2: 112 U3-rows + 5 U2-rows)

_f32 = dt.float32
_f32r = dt.float32r


def _build_program():
    """Build the per-core Bass program (identical across cores)."""
    nc = bass.Bass("TRN2", target_bir_lowering=False, debug=False)

    xrep_d = nc.dram_tensor("xrep", [BS, 128, C], _f32, kind="ExternalInput")
    trep_d = nc.dram_tensor("trep", [3, 128, BS, C], _f32r, kind="ExternalInput")
    wn1_d = nc.dram_tensor("wn1", [BS, C], _f32r, kind="ExternalInput")
    u1w_d = nc.dram_tensor("u1w", [1, WX2], _f32r, kind="ExternalInput")
    u3cat_d = nc.dram_tensor("u3cat", [3, 128, WXV], _f32r, kind="ExternalInput")
    sel6_d = nc.dram_tensor("sel6", [128, 6, WX2], _f32r, kind="ExternalInput")
    sel9_d = nc.dram_tensor("sel9", [WX2, EQ], _f32r, kind="ExternalInput")
    out_d = nc.dram_tensor("out", [BS, C * EQ], _f32, kind="ExternalOutput")

    with tile.TileContext(nc) as tc:
        with tc.tile_pool(name="const", bufs=1) as cpool:
            u3sb = cpool.tile([128, 3, WXV], _f32r)
            nc.sync.dma_start(out=u3sb[:], in_=u3cat_d[:].rearrange("j p f -> p j f"))
            sel6sb = cpool.tile([128, 6, WX2], _f32r)
            nc.sync.dma_start(out=sel6sb[:], in_=sel6_d[:])
            sel9sb = cpool.tile([WX2, EQ], _f32r)
            nc.sync.dma_start(out=sel9sb[:], in_=sel9_d[:])
            u1wsb = cpool.tile([1, WX2], _f32r)
            nc.sync.dma_start(out=u1wsb[:], in_=u1w_d[:])
            outsb = cpool.tile([C, BS * EQ], _f32)    # [c, (b, w)] staging

            # ---------------- main loop over node groups ----------------
            with tc.tile_pool(name="work", bufs=2) as pool, \
                 tc.tile_pool(name="ps_big", bufs=1, space="PSUM") as psb, \
                 tc.tile_pool(name="ps_c1", bufs=1, space="PSUM") as psc, \
                 tc.tile_pool(name="ps_o3", bufs=1, space="PSUM") as pso:
                for g in range(NG):
                    bsl = slice(g * G, (g + 1) * G)

                    xrep = pool.tile([128, F], _f32, tag="xrep")
                    nc.sync.dma_start(
                        out=xrep[:], in_=xrep_d[bsl].rearrange("b p c -> p b c")
                    )

                    # t4[p,(j,b,c)] = x[b,c,i(p)]*wn3[b,k(j,p),c] (host-built;
                    # chunk2 rows 112:117 carry wn2 for the folded U2 term)
                    t4 = pool.tile([128, 3, F], _f32r, tag="t4")
                    nc.sync.dma_start(
                        out=t4[:], in_=trep_d[:, :, bsl].rearrange("j p b c -> p j b c")
                    )

                    wn1sb = pool.tile([1, F], _f32r, tag="wn1sb")
                    nc.sync.dma_start(
                        out=wn1sb[:], in_=wn1_d[bsl].unsqueeze(0)
                    )

                    # main matmuls (two 3-bank PSUM halves for cross-group
                    # overlap) + m6 = out1T * x_v (v = p % 16) per half
                    m6 = pool.tile([128, 6, F], _f32r, tag="m6")
                    halves = []
                    for h in range(2):
                        ph = psb.tile([128, 3, F], _f32, tag=f"out1{h}")
                        halves.append(ph)
                        for mm in range(3):
                            m = 3 * h + mm
                            for j in range(3):
                                k = KCH[j]
                                nc.tensor.matmul(
                                    ph[:, mm, :],
                                    u3sb[:k, j, 128 * m : 128 * (m + 1)],
                                    t4[:k, j, :],
                                    start=(j == 0),
                                    stop=(j == 2),
                                )
                        nc.vector.tensor_mul(
                            m6[:, 3 * h : 3 * (h + 1), :],
                            ph[:],
                            xrep[:, None, :].to_broadcast([128, 3, F]),
                        )

                    c1_ps = psc.tile([WX2, F], _f32, tag="c1")

                    # 6 accumulating selector matmuls + U1*wn1 -> c1 [48, F]
                    for m in range(6):
                        nc.tensor.matmul(
                            c1_ps[:],
                            sel6sb[:, m, :],
                            m6[:, m, :],
                            start=(m == 0),
                            stop=False,
                        )
                    nc.tensor.matmul(
                        c1_ps[:],
                        u1wsb[:],
                        wn1sb[:],
                        start=False,
                        stop=True,
                    )

                    m9e = pool.tile([WX2, F], _f32r, tag="m9e")
                    nc.vector.tensor_mul(m9e[:], c1_ps[:], xrep[:WX2, :])

                    # final contraction per node: lhsT = m9e b-slice [48, C],
                    # rhs = sel9 [48, 3] -> out [c, 3]
                    o3_ps = pso.tile([C, G, EQ], _f32, tag="o3")
                    for b in range(G):
                        nc.tensor.matmul(
                            o3_ps[:, b, :],
                            m9e[:, C * b : C * (b + 1)].bitcast(_f32),
                            sel9sb[:].bitcast(_f32),
                            start=True,
                            stop=True,
                        )
                    nc.scalar.copy(
                        outsb[:, g * G * EQ : (g + 1) * G * EQ], o3_ps[:]
                    )

            # ---------------- end phase: layout transform ----------------
            with tc.tile_pool(name="fin", bufs=2) as fpool, \
                 tc.tile_pool(name="ps_fin", bufs=2, space="PSUM") as psf:
                ident128 = cpool.tile([128, 128], _f32)
                make_identity(nc, ident128[:])

                # [c, (b, w)] -> [b, (c, w)] via 3 big transposes
                finsb = fpool.tile([BS, C * EQ], _f32, tag="finsb")
                outsb_r = outsb[:].rearrange("c (b w) -> c b w", w=EQ)
                finsb_r = finsb[:].rearrange("b (c w) -> b c w", w=EQ)
                for w in range(EQ):
                    fin_ps = psf.tile([BS, C], _f32, tag="fin")
                    nc.tensor.transpose(fin_ps[:], outsb_r[:, :, w], ident128[:])
                    nc.scalar.copy(finsb_r[:, :, w], fin_ps[:])

                nc.sync.dma_start(out=out_d[:], in_=finsb[:])

    # Walrus codegen allows at most one sync-wait per instruction; Tile can
    # emit more. Split them exactly as Bacc.compile does.
    import bass_rust
    bass_rust.move_matmul_waits_to_ldweights(nc.m)
    bass_rust.generate_event_semaphores(nc)
    return nc


def _host_prep(x, y, U3, U2, U1, w_max, w2, w1):
    """Numpy-side input prep: gather per-element weights, transpose x,
    repack U3/U2, build selectors. Returns (shared_consts, per_core_fn)."""
    x = np.ascontiguousarray(x, dtype=np.float32)
    elem = np.argmax(y, axis=1)

    wn3 = w_max[elem]                       # [B, 23, C]
    wn1 = np.ascontiguousarray(w1[elem][:, 0, :])   # [B, C]

    # trep[j, p, b, c] = x[b, c, i(p)] * wn3[b, 8j + p//16, c]; chunk2 rows
    # 112:117 = wn2 (folded U2 contraction operand)
    trep = np.zeros((B, 3, 128, C), dtype=np.float32)
    wn3r = np.repeat(wn3, ELL, axis=1)      # [B, 368, C]
    xtile = np.tile(x.transpose(0, 2, 1), (1, P3, 1))  # [B, 368, C]
    trep.reshape(B, 384, C)[:, :368, :] = wn3r * xtile
    trep[:, 2, 112 : 112 + P2, :] = w2[elem]
    trep = np.ascontiguousarray(trep.transpose(1, 2, 0, 3))  # [3, 128, B, C]

    xT = x.transpose(0, 2, 1)                       # [B, 16, C]
    xrep = np.ascontiguousarray(np.tile(xT, (1, 8, 1)))  # [B, 128, C]

    # U3cat: [k, i, (w, x2, v)] chunks of 128; chunk2 rows 112:117 = U2
    u3k = U3.transpose(4, 3, 0, 1, 2).reshape(ELL * P3, WXV)  # [(k,i), wxv]
    u2k = U2.transpose(3, 0, 1, 2).reshape(P2, WXV)
    u3cat = np.zeros((3, 128, WXV), dtype=np.float32)
    u3cat[0] = u3k[0:128]
    u3cat[1] = u3k[128:256]
    u3cat[2, 0:112] = u3k[256:368]
    u3cat[2, 112 : 112 + P2] = u2k

    sel6 = np.zeros((128, 6, WX2), dtype=np.float32)
    for m in range(6):
        for p in range(128):
            sel6[p, m, 8 * m + p // 16] = 1.0

    sel9 = np.zeros((WX2, EQ), dtype=np.float32)
    for p in range(WX2):
        sel9[p, p // 16] = 1.0
    u1w = np.ascontiguousarray(U1[:, :, 0].reshape(1, WX2))

    shared = {"u3cat": u3cat, "sel6": sel6, "sel9": sel9, "u1w": u1w}

    def per_core(ci):
        s = slice(ci * BS, (ci + 1) * BS)
        m = {
            "xrep": np.ascontiguousarray(xrep[s]),
            "trep": np.ascontiguousarray(trep[:, :, s]),
            "wn1": np.ascontiguousarray(wn1[s]),
        }
        m.update(shared)
        return m

    return per_core


_PROGRAM_CACHE = {}


def kernel(**inputs) -> np.ndarray:
    from concourse.bass_utils import run_bass_kernel_spmd

    per_core = _host_prep(
        np.asarray(inputs["x"]), np.asarray(inputs["y"]),
        np.asarray(inputs["U3"]), np.asarray(inputs["U2"]),
        np.asarray(inputs["U1"]), np.asarray(inputs["w_max"]),
        np.asarray(inputs["w2"]), np.asarray(inputs["w1"]),
    )

    if "nc" not in _PROGRAM_CACHE:
        _PROGRAM_CACHE["nc"] = _build_program()
    nc = _PROGRAM_CACHE["nc"]

    in_maps = [per_core(ci) for ci in range(N_CORES)]
    res = run_bass_kernel_spmd(nc, in_maps, core_ids=list(range(N_CORES)))
    out = np.concatenate([r["out"] for r in res.results], axis=0)
    return out.astype(np.float32)


if __name__ == "__main__":
    # smoke test in CoreSim on core 0's shard
    from concourse.bass_interp import CoreSim

    rng = np.random.default_rng(0)
    x = rng.standard_normal((B, C, ELL)).astype(np.float32)
    elem = rng.integers(0, E, size=B)
    y = np.eye(E, dtype=np.float32)[elem]
    U3 = (rng.standard_normal((EQ, ELL, ELL, ELL, P3)) * 0.1).astype(np.float32)
    U2 = (rng.standard_normal((EQ, ELL, ELL, P2)) * 0.1).astype(np.float32)
    U1 = (rng.standard_normal((EQ, ELL, P1)) * 0.1).astype(np.float32)
    w_max = (rng.standard_normal((E, P3, C)) / P3).astype(np.float32)
    w2 = (rng.standard_normal((E, P2, C)) / P2).astype(np.float32)
    w1 = (rng.standard_normal((E, P1, C)) / P1).astype(np.float32)

    per_core = _host_prep(x, y, U3, U2, U1, w_max, w2, w1)
    nc = _build_program()
    sim = CoreSim(nc)
    m = per_core(0)
    for k, v in m.items():
        sim.tensor(k)[:] = v
    sim.simulate(check_with_hw=False, trace_hw=False)
    got = np.array(sim.tensor("out"))

    # numpy reference for core 0 shard
    def ref_np(x, y, U3, U2, U1, w_max, w2, w1):
        wn3 = np.einsum("be,ekc->bkc", y, w_max)
        t = np.einsum("bkc,bci->bcik", wn3, x)
        out = np.einsum("wxvik,bcik->bcwxv", U3, t)
        wn2 = np.einsum("be,ekc->bkc", y, w2)
        c2 = np.einsum("wxvk,bkc->bcwxv", U2, wn2) + out
        out = np.einsum("bcwxi,bci->bcwx", c2, x)
        wn1 = np.einsum("be,ekc->bkc", y, w1)
        c1 = np.einsum("wxk,bkc->bcwx", U1, wn1) + out
        out = np.einsum("bcwi,bci->bcw", c1, x)
        return out.reshape(out.shape[0], -1)

    want = ref_np(x[:BS], y[:BS], U3, U2, U1, w_max, w2, w1)
    err = np.abs(got - want).max() / (np.abs(want).max() + 1e-30)
    print(f"CoreSim vs numpy rel err: {err:.3e}")
    assert err < 2e-2, "FAIL"
    print("SIM PASS")
